# revision 16
# baseline (speedup 1.0000x reference)
"""Additive (Bahdanau-style) attention kernel for Trainium2, 8 NeuronCores.

reference computation (per batch b of 32, T=1024 timesteps, H=1024):
    mlp_hidden = selu([enc[b,t]; hid[b]] @ W1 + b1)     # (T, H)
    scores     = mlp_hidden @ W2 + b2                   # (T, 1)
    weights    = softmax(scores over t)
    out[b]     = sum_t weights[t] * enc[b,t]            # (H,)

Distribution: data-parallel over batch, 4 batches per core, no collectives.

Per-core algorithm (v3, fp8 DoubleRow, software-pipelined):
  - the hid @ W1[H:] + b1 term is per-batch constant; precomputed on the
    host and shipped as per-(j,b) bias columns (relu / exp variants).
  - scores path in fp8e4: E loaded once from HBM as bf16 (DMA cast),
    PE-transposed per 128x128 block, PSUM->SBUF copy casts to fp8.  W1a
    host-cast to fp8 with a 2^10 scale (entries ~1e-2 sit below fp8's
    normal range); the descale rides the ACT/DVE affine inputs.  The mlp
    matmul uses perf_mode=DoubleRow: one instruction contracts 256 rows.
  - selu = max(x,0) + min(alpha*e^x, alpha) (+const, dropped by softmax).
    exp always on ACT; the relu alternates between ACT (even j) and DVE
    tensor_scalar (odd j) to keep the j-loop from being ACT-bound.  For
    DVE j's the whole s2 is scaled by SW (host pre-scales that j's bias
    by SW and W2 column by 1/SW) so no extra scaling op is needed.
  - eT cast-copies alternate DVE / ACT per k for the same reason.
  - scores dot / context matmul have 1-wide outputs; packed 4-way into
    disjoint 32-column PE groups via tile_position (stationaries zero-
    padded to 32 cols so all PSUM partitions are written), then reduced
    across the 4 partial rows by a masked-ones PE matmul to partition 0.
  - software pipeline per batch b: loads(b+1) up front; transpose group
    tt of batch b+1 emitted after main-matmul group j=tt; epilogue(b-1)
    pieces emitted after j=4/5/6 so its matmuls land mid-stream (PE warm,
    no HAM re-throttle); score tail at the end of the j-loop.
  - softmax: exp off the reduced score rows (scores O(1), no max shift);
    1/Z folded into the output copy's scale.
"""

import math

import ml_dtypes
import numpy as np

import concourse.tile as tile
from concourse.masks import make_identity
from concourse import bacc, mybir
from concourse.bass_utils import run_bass_kernel_spmd

F32 = mybir.dt.float32
BF16 = mybir.dt.bfloat16
FP8 = mybir.dt.float8e4
ALU = mybir.AluOpType
ACTF = mybir.ActivationFunctionType
DR = mybir.MatmulPerfMode.DoubleRow

N_CORES = 8
B = 32
T = 1024
H = 1024
BL = B // N_CORES          # batches per core = 4
KC = H // 128              # contraction chunks = 8
JC = H // 128              # hidden-unit chunks = 8
TT = T // 128              # 128-row t-subtiles per batch = 8

SELU_LAMBDA = 1.0507009873554805
SELU_ALPHA = 1.6732632423543772
SW = 1024.0                # fp8 scale for W1a (and odd-j s2 scale)


def build_kernel():
    nc = bacc.Bacc("TRN2", target_bir_lowering=False, debug=False,
                   num_devices=N_CORES)

    enc = nc.dram_tensor("enc", [BL, TT, 128, H], F32, kind="ExternalInput").ap()
    w1a8 = nc.dram_tensor("w1a8", [JC, KC, 128, 128], FP8,
                          kind="ExternalInput").ap()
    w2lp = nc.dram_tensor("w2lp", [128, JC, 32], BF16, kind="ExternalInput").ap()
    hbe = nc.dram_tensor("hbe", [128, JC, BL], F32, kind="ExternalInput").ap()
    hbr = nc.dram_tensor("hbr", [128, JC, BL], F32, kind="ExternalInput").ap()
    maskb = nc.dram_tensor("maskb", [128, 2], BF16, kind="ExternalInput").ap()
    wcz = nc.dram_tensor("wcz", [128, KC, 32], BF16, kind="ExternalInput").ap()
    out = nc.dram_tensor("out", [BL, H], F32, kind="ExternalOutput").ap()
    zs = nc.dram_tensor("zs", [BL, 2], F32, kind="ExternalOutput").ap()

    with tile.TileContext(nc) as tc:
        with (
            tc.tile_pool(name="consts", bufs=1) as consts,
            tc.tile_pool(name="encp", bufs=3) as encp,
            tc.tile_pool(name="etp", bufs=2) as etp,
            tc.tile_pool(name="selu", bufs=4) as selup,
            tc.tile_pool(name="score", bufs=2) as scorep,
            tc.tile_pool(name="outp", bufs=2) as outp,
            tc.tile_pool(name="psum", bufs=2, space="PSUM") as psum,
        ):
            # identity + PE warmup first: the warmup keeps the TensorE
            # activity monitor busy (so the clock ungates) while the first
            # batch's DMAs stream in.
            identity = consts.tile([128, 128], BF16)
            make_identity(nc, identity)
            one1 = consts.tile([1, 1], F32)
            nc.vector.memset(one1, 1.0)
            junk = consts.tile([128, 128], BF16)
            nc.vector.memset(junk, 0.0)
            warm_ps = psum.tile([128, 128], BF16, tag="trans", bufs=2)
            for _ in range(14):
                nc.tensor.transpose(warm_ps, junk, junk)

            # --- batch 0 + weights prologue -----------------------------
            # gpsimd (the only casting DMA queue) wakes at ~10us and then
            # streams ~2.3us/tile; the two hardware queues wake at ~6us
            # but only manage ~8.5us/tile.  Batch 0 is split 5/2/1 so all
            # three finish around the same time; hw-loaded tiles arrive as
            # raw f32 and ACT/DVE (idle during the fill) cast them.  W1a
            # is loaded in per-j slices so the first DoubleRow matmul does
            # not wait for the whole megabyte.
            e_ts = []
            e32s = {}
            for tt in range(TT):
                e_ts.append(encp.tile([128, H], BF16, tag="e", bufs=3 * TT,
                                      name=f"e_0_{tt}"))
            for tt in (5, 6, 7):
                e32s[tt] = encp.tile([128, H], F32, tag="e32", bufs=3,
                                     name=f"e32_{tt}")
            hbe_sb = consts.tile([128, JC, BL], F32)
            hbr_sb = consts.tile([128, JC, BL], F32)
            w2lp_sb = consts.tile([128, JC, 32], BF16)
            maskb_sb = consts.tile([128, 2], BF16)
            wcol_pad = consts.tile([128, KC, 32], BF16)
            w1a_sb = consts.tile([128, KC, H], FP8)
            for tt in range(5):
                nc.gpsimd.dma_start(out=e_ts[tt], in_=enc[0, tt])
            nc.sync.dma_start(out=e32s[5], in_=enc[0, 5])
            nc.sync.dma_start(out=e32s[6], in_=enc[0, 6])
            nc.scalar.dma_start(out=hbe_sb, in_=hbe)
            nc.scalar.dma_start(out=hbr_sb, in_=hbr)
            nc.scalar.dma_start(out=e32s[7], in_=enc[0, 7])
            for j in range(JC):
                nc.scalar.dma_start(
                    out=w1a_sb[:, :, j * 128:(j + 1) * 128],
                    in_=w1a8[j].rearrange("k p jj -> p k jj"))
            nc.scalar.dma_start(out=w2lp_sb, in_=w2lp)
            nc.scalar.dma_start(out=maskb_sb, in_=maskb)
            # zero-padded context-weight stationary: zeros come from the
            # host; each epilogue overwrites only column 0 of each block.
            nc.scalar.dma_start(out=wcol_pad, in_=wcz)
            # f32 -> bf16 casts for the hw-loaded tiles
            nc.vector.tensor_copy(out=e_ts[5], in_=e32s[5])
            nc.scalar.activation(out=e_ts[6], in_=e32s[6],
                                 func=ACTF.Copy, scale=1.0)
            nc.vector.tensor_copy(out=e_ts[7], in_=e32s[7])

            def emit_loads(b):
                # gpsimd software-DGE DMAs cast f32 -> bf16 on the fly.
                e_ts = []
                for tt in range(TT):
                    e_t = encp.tile([128, H], BF16, tag="e", bufs=3 * TT,
                                    name=f"e_{b}_{tt}")
                    nc.gpsimd.dma_start(out=e_t, in_=enc[b, tt])
                    e_ts.append(e_t)
                return e_ts

            def alloc_eT(b):
                return etp.tile([128, KC, T], FP8, tag="eT", bufs=2,
                                name=f"eT_{b}")

            def emit_transpose_tt(b, e_ts, eT, tt):
                # all KC h-chunks of one t-subtile -> eT[:, :, tt*128:...]
                # (starts as soon as e_ts[tt]'s DMA lands).  The cast copy
                # alternates DVE/ACT to balance engine load.
                tp = psum.tile([128, KC, 128], BF16, tag="trans", bufs=2)
                for k in range(KC):
                    nc.tensor.transpose(
                        tp[:, k, :],
                        e_ts[tt][:, k * 128:(k + 1) * 128],
                        identity,
                    )
                dst = eT[:, :, tt * 128:(tt + 1) * 128]
                src = tp
                if tt % 2 == 0:
                    nc.vector.tensor_copy(out=dst, in_=src)
                else:
                    nc.scalar.activation(out=dst, in_=src, func=ACTF.Copy,
                                         scale=1.0)

            def emit_scores_tail(b, sc_ps):
                # stage partials in SBUF (bf16), PE-reduce per tg to
                # partition 0, exp with row-sum accumulation.
                scs = scorep.tile([128, 512], BF16, tag="scs")
                nc.scalar.activation(out=scs, in_=sc_ps, func=ACTF.Copy,
                                     scale=1.0)
                expw = scorep.tile([1, T], F32, tag="expw")
                rs2 = scorep.tile([1, 2], F32, tag="rsum2")
                for tg in range(2):
                    scr = psum.tile([1, 512], F32, tag="ctx", bufs=1)
                    nc.tensor.matmul(
                        scr,
                        lhsT=maskb_sb[:, tg:tg + 1],
                        rhs=scs,
                        start=True, stop=True,
                    )
                    nc.scalar.activation(
                        out=expw[:, tg * 512:(tg + 1) * 512], in_=scr,
                        func=ACTF.Exp, scale=1.0, accum_out=rs2[:, tg:tg + 1])
                return expw, rs2

            # ---- epilogue pieces (for batch whose phase1 has finished) --
            def epi_weights(state):
                # weights -> padded column stationary (PE transpose).
                e_ts, expw = state[:2]
                w_ps = psum.tile([128, KC, 1], F32, tag="ctx", bufs=1)
                for c in range(KC):
                    nc.tensor.transpose(
                        w_ps[:, c, :],
                        expw[0:1, c * 128:(c + 1) * 128],
                        one1,
                    )
                nc.vector.tensor_copy(out=wcol_pad[:, :, 0:1], in_=w_ps)

            def epi_context(state):
                # context[h] partials, col-group packed 4-way.
                e_ts = state[0]
                cp = psum.tile([128, 512], F32, tag="ctx", bufs=1)
                for half in range(2):
                    for tch in range(KC):
                        pos = 64 * (tch % 2) + 32 * half
                        nc.tensor.matmul(
                            cp[pos:pos + 32, :],
                            lhsT=wcol_pad[:, tch, :],
                            rhs=e_ts[tch][:, half * 512:(half + 1) * 512],
                            start=(tch < 2),
                            stop=(tch >= KC - 2),
                            tile_position=(0, pos),
                        )
                return cp

            def epi_out(b, cp, state):
                # unnormalized context out; softmax Z rides along in zs and
                # the host divides (removes the rs->recip->scale chain).
                rs2 = state[2]
                cps = outp.tile([128, 512], BF16, tag="cps")
                nc.scalar.activation(out=cps, in_=cp, func=ACTF.Copy,
                                     scale=1.0)
                ob = outp.tile([1, H], F32, tag="ob")
                for half in range(2):
                    ctxr = psum.tile([1, 512], F32, tag="ctx", bufs=1)
                    nc.tensor.matmul(
                        ctxr,
                        lhsT=maskb_sb[:, half:half + 1],
                        rhs=cps,
                        start=True, stop=True,
                    )
                    dst = ob[:, half * 512:(half + 1) * 512]
                    if half == 0:
                        nc.vector.tensor_copy(out=dst, in_=ctxr)
                    else:
                        nc.scalar.activation(out=dst, in_=ctxr,
                                             func=ACTF.Copy, scale=1.0)
                nc.sync.dma_start(out=out[b:b + 1, :], in_=ob)
                nc.sync.dma_start(out=zs[b:b + 1, :], in_=rs2)

            def phase1(b, e_ts, eT, next_ctx, prev_ctx):
                """Main pass for batch b.

                next_ctx: (e_ts, eT) of batch b+1 whose transposes are
                interleaved into this j-loop (or None).
                prev_ctx: (b-1, state) whose epilogue is interleaved
                (or None).
                """
                sc_ps = psum.tile([128, 512], F32, tag="sc", bufs=1)
                s2_prev = None
                epi = {}

                def emit_score(j, s2):
                    for tg in range(2):
                        pos = 64 * (j % 2) + 32 * tg
                        nc.tensor.matmul(
                            sc_ps[pos:pos + 32, :],
                            lhsT=w2lp_sb[:, j, :],
                            rhs=s2[:, tg * 512:(tg + 1) * 512],
                            start=(j < 2),
                            stop=(j >= JC - 2),
                            tile_position=(0, pos),
                        )

                for j in range(JC):
                    mp = psum.tile([128, T], F32, tag="mlp", bufs=2)
                    for tg in range(2):
                        for kk in range(KC // 2):
                            nc.tensor.matmul(
                                mp[:, tg * 512:(tg + 1) * 512],
                                lhsT=w1a_sb[:, 2 * kk:2 * kk + 2,
                                            j * 128:(j + 1) * 128],
                                rhs=eT[:, 2 * kk:2 * kk + 2,
                                       tg * 512:(tg + 1) * 512],
                                start=(kk == 0),
                                stop=(kk == KC // 2 - 1),
                                perf_mode=DR,
                            )
                    if next_ctx is not None:
                        # front-loaded so the last cast-copy lands before
                        # the next batch's first DoubleRow matmul.
                        for tt in ([j] if j < 5 else [5, 6] if j == 5
                                   else [7] if j == 6 else []):
                            emit_transpose_tt(b + 1, next_ctx[0],
                                              next_ctx[1], tt)
                    if prev_ctx is not None:
                        pb, pstate = prev_ctx
                        if j == 4:
                            epi_weights(pstate)
                        elif j == 5:
                            epi["cp"] = epi_context(pstate)
                        elif j == 6:
                            epi_out(pb, epi["cp"], pstate)
                    if s2_prev is not None:
                        emit_score(j - 1, s2_prev)
                    e2 = selup.tile([128, T], BF16, tag="e2")
                    nc.scalar.activation(out=e2, in_=mp, func=ACTF.Exp,
                                         bias=hbe_sb[:, j, b:b + 1],
                                         scale=1.0 / SW)
                    r2 = selup.tile([128, T], BF16, tag="r2")
                    if j % 2 == 0:
                        # ACT path: r2 = relu(mp/SW + hb)
                        nc.scalar.activation(out=r2, in_=mp, func=ACTF.Relu,
                                             bias=hbr_sb[:, j, b:b + 1],
                                             scale=1.0 / SW)
                        alpha_cap = SELU_ALPHA
                    else:
                        # DVE path, SW-scaled: r2 = max(mp + SW*hb, 0);
                        # this j's whole s2 is scaled by SW (host divides
                        # the W2 column by SW and offsets the exp bias).
                        nc.vector.tensor_scalar(
                            out=r2, in0=mp, scalar1=hbr_sb[:, j, b:b + 1],
                            scalar2=0.0, op0=ALU.add, op1=ALU.max,
                        )
                        alpha_cap = SELU_ALPHA * SW
                    # s2 = min(e2, alpha) + r2, single fused DVE op
                    s2 = selup.tile([128, T], BF16, tag="s2", bufs=3)
                    nc.vector.scalar_tensor_tensor(
                        out=s2, in0=e2, scalar=alpha_cap, in1=r2,
                        op0=ALU.min, op1=ALU.add,
                    )
                    s2_prev = s2
                emit_score(JC - 1, s2_prev)
                expw, rs2 = emit_scores_tail(b, sc_ps)
                return (e_ts, expw, rs2)

            # ---------------- top-level software pipeline ----------------
            eT = alloc_eT(0)
            for tt in (5, 0, 1, 7, 2, 3, 6, 4):
                emit_transpose_tt(0, e_ts, eT, tt)

            prev_state = None
            for b in range(BL):
                if b + 1 < BL:
                    e_ts_n = emit_loads(b + 1)
                    next_ctx = (e_ts_n, alloc_eT(b + 1))
                else:
                    next_ctx = None
                prev_ctx = (b - 1, prev_state) if prev_state is not None \
                    else None
                state = phase1(b, e_ts, eT, next_ctx, prev_ctx)
                prev_state = state
                if next_ctx is not None:
                    e_ts, eT = next_ctx

            # final epilogue (nothing left to hide it behind)
            epi_weights(prev_state)
            cp = epi_context(prev_state)
            epi_out(BL - 1, cp, prev_state)

    nc.compile()
    return nc


_NC_CACHE = None


def _get_nc():
    global _NC_CACHE
    if _NC_CACHE is None:
        _NC_CACHE = build_kernel()
    return _NC_CACHE


def make_in_maps(encoder_outputs, hidden_state, W1, b1, W2):
    enc = np.ascontiguousarray(np.asarray(encoder_outputs, np.float32))
    hid = np.ascontiguousarray(np.asarray(hidden_state, np.float32))
    W1 = np.asarray(W1, np.float32)
    b1 = np.asarray(b1, np.float32)
    W2 = np.asarray(W2, np.float32)

    bf16 = ml_dtypes.bfloat16
    f8 = ml_dtypes.float8_e4m3
    # cast to the HW e4m3 format, but ship the bytes under the e4m3fn
    # container dtype: the PJRT path rejects the IEEE f8E4M3 HLO type
    # while accepting f8E4M3FN, and bass's input check is fuzzy across
    # the two.
    w1a8 = np.ascontiguousarray(
        (W1[:H] * SW).reshape(KC, 128, JC, 128).transpose(2, 0, 1, 3)
    ).astype(f8).view(ml_dtypes.float8_e4m3fn)

    # per-j s2 scale: odd j's selu output is scaled by SW (DVE relu path)
    jscale = np.where(np.arange(JC) % 2 == 1, SW, 1.0).astype(np.float32)
    w2l = (W2[:, 0] * SELU_LAMBDA).reshape(JC, 128) / jscale[:, None]
    w2lp = np.zeros((128, JC, 32), bf16)
    w2lp[:, :, 0] = w2l.T.astype(bf16)

    # reduction masks: tg0/half0 partials live at partitions [0,32) and
    # [64,96) (real rows 0 and 64, zeros elsewhere), tg1/half1 at the
    # complement.
    m = np.zeros((128, 2), np.float32)
    m[0:32, 0] = 1.0
    m[64:96, 0] = 1.0
    m[32:64, 1] = 1.0
    m[96:128, 1] = 1.0

    # host-side hidden-state contribution: hb[b, :] = hid[b] @ W1[H:] + b1
    hb_all = hid[0] @ W1[H:] + b1                       # (B, H) f32
    ln_alpha = math.log(SELU_ALPHA)
    ln_sw = math.log(SW)

    in_maps = []
    for c in range(N_CORES):
        sl = slice(BL * c, BL * (c + 1))
        hb = hb_all[sl].reshape(BL, JC, 128).transpose(2, 1, 0)  # (128,JC,BL)
        # exp bias: hb + ln(alpha) (+ ln(SW) for odd j so e2 = SW*alpha*e^x)
        hbe = hb + ln_alpha + ln_sw * (np.arange(JC) % 2)[None, :, None]
        # relu bias: hb (ACT, even j) or SW*hb (DVE, odd j)
        hbr = hb * np.where(np.arange(JC) % 2 == 1, SW, 1.0)[None, :, None]
        in_maps.append({
            "enc": np.ascontiguousarray(enc[sl]).reshape(BL, TT, 128, H),
            "w1a8": w1a8,
            "w2lp": w2lp,
            "hbe": np.ascontiguousarray(hbe.astype(np.float32)),
            "hbr": np.ascontiguousarray(hbr.astype(np.float32)),
            "maskb": m.astype(bf16),
            "wcz": np.zeros((128, KC, 32), bf16),
        })
    return in_maps


def kernel(encoder_outputs, hidden_state, W1, b1, W2, b2):
    # b2 shifts every score equally; softmax is shift-invariant, so it is
    # deliberately unused.
    in_maps = make_in_maps(encoder_outputs, hidden_state, W1, b1, W2)
    nc = _get_nc()
    res = run_bass_kernel_spmd(nc, in_maps, core_ids=list(range(N_CORES)))
    out = np.empty((1, B, H), np.float32)
    for c in range(N_CORES):
        z = res.results[c]["zs"].sum(axis=1, keepdims=True)   # (BL, 1)
        out[0, BL * c:BL * (c + 1)] = res.results[c]["out"] / z
    return out


# revision 17
# speedup vs baseline: 1.2246x; 1.2246x over previous
"""Additive (Bahdanau-style) attention kernel for Trainium2, 8 NeuronCores.

reference computation (per batch b of 32, T=1024 timesteps, H=1024):
    mlp_hidden = selu([enc[b,t]; hid[b]] @ W1 + b1)     # (T, H)
    scores     = mlp_hidden @ W2 + b2                   # (T, 1)
    weights    = softmax(scores over t)
    out[b]     = sum_t weights[t] * enc[b,t]            # (H,)

Distribution: data-parallel over batch, 4 batches per core, no collectives.

Per-core algorithm (v3, fp8 DoubleRow, software-pipelined):
  - the hid @ W1[H:] + b1 term is per-batch constant; precomputed on the
    host and shipped as per-(j,b) bias columns (relu / exp variants).
  - scores path in fp8e4: E loaded once from HBM as bf16 (DMA cast),
    PE-transposed per 128x128 block, PSUM->SBUF copy casts to fp8.  W1a
    host-cast to fp8 with a 2^10 scale (entries ~1e-2 sit below fp8's
    normal range); the descale rides the ACT/DVE affine inputs.  The mlp
    matmul uses perf_mode=DoubleRow: one instruction contracts 256 rows.
  - selu = max(x,0) + min(alpha*e^x, alpha) (+const, dropped by softmax).
    exp always on ACT; the relu alternates between ACT (even j) and DVE
    tensor_scalar (odd j) to keep the j-loop from being ACT-bound.  For
    DVE j's the whole s2 is scaled by SW (host pre-scales that j's bias
    by SW and W2 column by 1/SW) so no extra scaling op is needed.
  - eT cast-copies alternate DVE / ACT per k for the same reason.
  - scores dot / context matmul have 1-wide outputs; packed 4-way into
    disjoint 32-column PE groups via tile_position (stationaries zero-
    padded to 32 cols so all PSUM partitions are written), then reduced
    across the 4 partial rows by a masked-ones PE matmul to partition 0.
  - software pipeline per batch b: loads(b+1) up front; transpose group
    tt of batch b+1 emitted after main-matmul group j=tt; epilogue(b-1)
    pieces emitted after j=4/5/6 so its matmuls land mid-stream (PE warm,
    no HAM re-throttle); score tail at the end of the j-loop.
  - softmax: exp off the reduced score rows (scores O(1), no max shift);
    1/Z folded into the output copy's scale.
"""

import math

import ml_dtypes
import numpy as np

import concourse.tile as tile
from concourse.masks import make_identity
from concourse import bacc, mybir
from concourse.bass_utils import run_bass_kernel_spmd

F32 = mybir.dt.float32
BF16 = mybir.dt.bfloat16
FP8 = mybir.dt.float8e4
ALU = mybir.AluOpType
ACTF = mybir.ActivationFunctionType
DR = mybir.MatmulPerfMode.DoubleRow

N_CORES = 8
B = 32
T = 1024
H = 1024
BL = B // N_CORES          # batches per core = 4
KC = H // 128              # contraction chunks = 8
JC = H // 128              # hidden-unit chunks = 8
TT = T // 128              # 128-row t-subtiles per batch = 8

SELU_LAMBDA = 1.0507009873554805
SELU_ALPHA = 1.6732632423543772
SW = 1024.0                # fp8 scale for W1a (and odd-j s2 scale)


def build_kernel():
    nc = bacc.Bacc("TRN2", target_bir_lowering=False, debug=False,
                   num_devices=N_CORES)

    enc = nc.dram_tensor("enc", [BL, TT, 128, H], F32, kind="ExternalInput").ap()
    w1a8 = nc.dram_tensor("w1a8", [KC, 128, H], FP8, kind="ExternalInput").ap()
    w2lp = nc.dram_tensor("w2lp", [128, JC, 32], BF16, kind="ExternalInput").ap()
    hbe = nc.dram_tensor("hbe", [128, JC, BL], F32, kind="ExternalInput").ap()
    hbr = nc.dram_tensor("hbr", [128, JC, BL], F32, kind="ExternalInput").ap()
    maskb = nc.dram_tensor("maskb", [128, 2], BF16, kind="ExternalInput").ap()
    wcz = nc.dram_tensor("wcz", [128, KC, 32], BF16, kind="ExternalInput").ap()
    out = nc.dram_tensor("out", [BL, H], F32, kind="ExternalOutput").ap()
    zs = nc.dram_tensor("zs", [BL, 2], F32, kind="ExternalOutput").ap()

    with tile.TileContext(nc) as tc:
        with (
            tc.tile_pool(name="consts", bufs=1) as consts,
            tc.tile_pool(name="encp", bufs=3) as encp,
            tc.tile_pool(name="etp", bufs=2) as etp,
            tc.tile_pool(name="selu", bufs=4) as selup,
            tc.tile_pool(name="score", bufs=2) as scorep,
            tc.tile_pool(name="outp", bufs=2) as outp,
            tc.tile_pool(name="psum", bufs=2, space="PSUM") as psum,
        ):
            # identity + PE warmup first: the warmup keeps the TensorE
            # activity monitor busy (so the clock ungates) while the first
            # batch's DMAs stream in.
            identity = consts.tile([128, 128], BF16)
            make_identity(nc, identity)
            one1 = consts.tile([1, 1], F32)
            nc.vector.memset(one1, 1.0)
            junk = consts.tile([128, 128], BF16)
            nc.vector.memset(junk, 0.0)
            warm_ps = psum.tile([128, 128], BF16, tag="trans", bufs=2)
            for _ in range(28):
                nc.tensor.transpose(warm_ps, junk, junk)

            # --- replicated weights / biases (sync queue, parallel with
            # the gpsimd encoder stream) --------------------------------
            hbe_sb = consts.tile([128, JC, BL], F32)
            nc.sync.dma_start(out=hbe_sb, in_=hbe)
            hbr_sb = consts.tile([128, JC, BL], F32)
            nc.sync.dma_start(out=hbr_sb, in_=hbr)
            w1a_sb = consts.tile([128, KC, H], FP8)
            nc.sync.dma_start(out=w1a_sb, in_=w1a8.rearrange("k p h -> p k h"))
            w2lp_sb = consts.tile([128, JC, 32], BF16)
            nc.sync.dma_start(out=w2lp_sb, in_=w2lp)
            maskb_sb = consts.tile([128, 2], BF16)
            nc.sync.dma_start(out=maskb_sb, in_=maskb)
            # zero-padded context-weight stationary: zeros come from the
            # host; each epilogue overwrites only column 0 of each block.
            wcol_pad = consts.tile([128, KC, 32], BF16)
            nc.sync.dma_start(out=wcol_pad, in_=wcz)

            def emit_loads(b):
                # gpsimd software-DGE DMAs cast f32 -> bf16 on the fly.
                e_ts = []
                for tt in range(TT):
                    e_t = encp.tile([128, H], BF16, tag="e", bufs=3 * TT,
                                    name=f"e_{b}_{tt}")
                    nc.gpsimd.dma_start(out=e_t, in_=enc[b, tt])
                    e_ts.append(e_t)
                return e_ts

            def alloc_eT(b):
                return etp.tile([128, KC, T], FP8, tag="eT", bufs=2,
                                name=f"eT_{b}")

            def emit_transpose_tt(b, e_ts, eT, tt):
                # all KC h-chunks of one t-subtile -> eT[:, :, tt*128:...]
                # (starts as soon as e_ts[tt]'s DMA lands).  The cast copy
                # alternates DVE/ACT to balance engine load.
                tp = psum.tile([128, KC, 128], BF16, tag="trans", bufs=2)
                for k in range(KC):
                    nc.tensor.transpose(
                        tp[:, k, :],
                        e_ts[tt][:, k * 128:(k + 1) * 128],
                        identity,
                    )
                dst = eT[:, :, tt * 128:(tt + 1) * 128]
                src = tp
                if tt % 2 == 0:
                    nc.vector.tensor_copy(out=dst, in_=src)
                else:
                    nc.scalar.activation(out=dst, in_=src, func=ACTF.Copy,
                                         scale=1.0)

            def emit_scores_tail(b, sc_ps):
                # stage partials in SBUF (bf16), PE-reduce per tg to
                # partition 0, exp with row-sum accumulation.
                scs = scorep.tile([128, 512], BF16, tag="scs")
                nc.scalar.activation(out=scs, in_=sc_ps, func=ACTF.Copy,
                                     scale=1.0)
                expw = scorep.tile([1, T], F32, tag="expw")
                rs2 = scorep.tile([1, 2], F32, tag="rsum2")
                for tg in range(2):
                    scr = psum.tile([1, 512], F32, tag="ctx", bufs=1)
                    nc.tensor.matmul(
                        scr,
                        lhsT=maskb_sb[:, tg:tg + 1],
                        rhs=scs,
                        start=True, stop=True,
                    )
                    nc.scalar.activation(
                        out=expw[:, tg * 512:(tg + 1) * 512], in_=scr,
                        func=ACTF.Exp, scale=1.0, accum_out=rs2[:, tg:tg + 1])
                return expw, rs2

            # ---- epilogue pieces (for batch whose phase1 has finished) --
            def epi_weights(state):
                # weights -> padded column stationary (PE transpose).
                e_ts, expw = state[:2]
                w_ps = psum.tile([128, KC, 1], F32, tag="ctx", bufs=1)
                for c in range(KC):
                    nc.tensor.transpose(
                        w_ps[:, c, :],
                        expw[0:1, c * 128:(c + 1) * 128],
                        one1,
                    )
                nc.vector.tensor_copy(out=wcol_pad[:, :, 0:1], in_=w_ps)

            def epi_context(state):
                # context[h] partials, col-group packed 4-way.
                e_ts = state[0]
                cp = psum.tile([128, 512], F32, tag="ctx", bufs=1)
                for half in range(2):
                    for tch in range(KC):
                        pos = 64 * (tch % 2) + 32 * half
                        nc.tensor.matmul(
                            cp[pos:pos + 32, :],
                            lhsT=wcol_pad[:, tch, :],
                            rhs=e_ts[tch][:, half * 512:(half + 1) * 512],
                            start=(tch < 2),
                            stop=(tch >= KC - 2),
                            tile_position=(0, pos),
                        )
                return cp

            def epi_out(b, cp, state):
                # unnormalized context out; softmax Z rides along in zs and
                # the host divides (removes the rs->recip->scale chain).
                rs2 = state[2]
                cps = outp.tile([128, 512], BF16, tag="cps")
                nc.scalar.activation(out=cps, in_=cp, func=ACTF.Copy,
                                     scale=1.0)
                ob = outp.tile([1, H], F32, tag="ob")
                for half in range(2):
                    ctxr = psum.tile([1, 512], F32, tag="ctx", bufs=1)
                    nc.tensor.matmul(
                        ctxr,
                        lhsT=maskb_sb[:, half:half + 1],
                        rhs=cps,
                        start=True, stop=True,
                    )
                    dst = ob[:, half * 512:(half + 1) * 512]
                    if half == 0:
                        nc.vector.tensor_copy(out=dst, in_=ctxr)
                    else:
                        nc.scalar.activation(out=dst, in_=ctxr,
                                             func=ACTF.Copy, scale=1.0)
                nc.sync.dma_start(out=out[b:b + 1, :], in_=ob)
                nc.sync.dma_start(out=zs[b:b + 1, :], in_=rs2)

            def phase1(b, e_ts, eT, next_ctx, prev_ctx):
                """Main pass for batch b.

                next_ctx: (e_ts, eT) of batch b+1 whose transposes are
                interleaved into this j-loop (or None).
                prev_ctx: (b-1, state) whose epilogue is interleaved
                (or None).
                """
                sc_ps = psum.tile([128, 512], F32, tag="sc", bufs=1)
                s2_prev = None
                epi = {}

                def emit_score(j, s2):
                    for tg in range(2):
                        pos = 64 * (j % 2) + 32 * tg
                        nc.tensor.matmul(
                            sc_ps[pos:pos + 32, :],
                            lhsT=w2lp_sb[:, j, :],
                            rhs=s2[:, tg * 512:(tg + 1) * 512],
                            start=(j < 2),
                            stop=(j >= JC - 2),
                            tile_position=(0, pos),
                        )

                for j in range(JC):
                    mp = psum.tile([128, T], F32, tag="mlp", bufs=2)
                    for tg in range(2):
                        for kk in range(KC // 2):
                            nc.tensor.matmul(
                                mp[:, tg * 512:(tg + 1) * 512],
                                lhsT=w1a_sb[:, 2 * kk:2 * kk + 2,
                                            j * 128:(j + 1) * 128],
                                rhs=eT[:, 2 * kk:2 * kk + 2,
                                       tg * 512:(tg + 1) * 512],
                                start=(kk == 0),
                                stop=(kk == KC // 2 - 1),
                                perf_mode=DR,
                            )
                    if next_ctx is not None:
                        # front-loaded so the last cast-copy lands before
                        # the next batch's first DoubleRow matmul.
                        for tt in ([j] if j < 5 else [5, 6] if j == 5
                                   else [7] if j == 6 else []):
                            emit_transpose_tt(b + 1, next_ctx[0],
                                              next_ctx[1], tt)
                    if prev_ctx is not None:
                        pb, pstate = prev_ctx
                        if j == 4:
                            epi_weights(pstate)
                        elif j == 5:
                            epi["cp"] = epi_context(pstate)
                        elif j == 6:
                            epi_out(pb, epi["cp"], pstate)
                    if s2_prev is not None:
                        emit_score(j - 1, s2_prev)
                    e2 = selup.tile([128, T], BF16, tag="e2")
                    nc.scalar.activation(out=e2, in_=mp, func=ACTF.Exp,
                                         bias=hbe_sb[:, j, b:b + 1],
                                         scale=1.0 / SW)
                    r2 = selup.tile([128, T], BF16, tag="r2")
                    if j % 2 == 0:
                        # ACT path: r2 = relu(mp/SW + hb)
                        nc.scalar.activation(out=r2, in_=mp, func=ACTF.Relu,
                                             bias=hbr_sb[:, j, b:b + 1],
                                             scale=1.0 / SW)
                        alpha_cap = SELU_ALPHA
                    else:
                        # DVE path, SW-scaled: r2 = max(mp + SW*hb, 0);
                        # this j's whole s2 is scaled by SW (host divides
                        # the W2 column by SW and offsets the exp bias).
                        nc.vector.tensor_scalar(
                            out=r2, in0=mp, scalar1=hbr_sb[:, j, b:b + 1],
                            scalar2=0.0, op0=ALU.add, op1=ALU.max,
                        )
                        alpha_cap = SELU_ALPHA * SW
                    # s2 = min(e2, alpha) + r2, single fused DVE op
                    s2 = selup.tile([128, T], BF16, tag="s2", bufs=3)
                    nc.vector.scalar_tensor_tensor(
                        out=s2, in0=e2, scalar=alpha_cap, in1=r2,
                        op0=ALU.min, op1=ALU.add,
                    )
                    s2_prev = s2
                emit_score(JC - 1, s2_prev)
                expw, rs2 = emit_scores_tail(b, sc_ps)
                return (e_ts, expw, rs2)

            # ---------------- top-level software pipeline ----------------
            e_ts = emit_loads(0)
            eT = alloc_eT(0)
            for tt in range(TT):
                emit_transpose_tt(0, e_ts, eT, tt)

            prev_state = None
            for b in range(BL):
                if b + 1 < BL:
                    e_ts_n = emit_loads(b + 1)
                    next_ctx = (e_ts_n, alloc_eT(b + 1))
                else:
                    next_ctx = None
                prev_ctx = (b - 1, prev_state) if prev_state is not None \
                    else None
                state = phase1(b, e_ts, eT, next_ctx, prev_ctx)
                prev_state = state
                if next_ctx is not None:
                    e_ts, eT = next_ctx

            # final epilogue (nothing left to hide it behind)
            epi_weights(prev_state)
            cp = epi_context(prev_state)
            epi_out(BL - 1, cp, prev_state)

    nc.compile()
    return nc


_NC_CACHE = None


def _get_nc():
    global _NC_CACHE
    if _NC_CACHE is None:
        _NC_CACHE = build_kernel()
    return _NC_CACHE


def make_in_maps(encoder_outputs, hidden_state, W1, b1, W2):
    enc = np.ascontiguousarray(np.asarray(encoder_outputs, np.float32))
    hid = np.ascontiguousarray(np.asarray(hidden_state, np.float32))
    W1 = np.asarray(W1, np.float32)
    b1 = np.asarray(b1, np.float32)
    W2 = np.asarray(W2, np.float32)

    bf16 = ml_dtypes.bfloat16
    f8 = ml_dtypes.float8_e4m3
    # cast to the HW e4m3 format, but ship the bytes under the e4m3fn
    # container dtype: the PJRT path rejects the IEEE f8E4M3 HLO type
    # while accepting f8E4M3FN, and bass's input check is fuzzy across
    # the two.
    w1a8 = np.ascontiguousarray(
        (W1[:H] * SW).reshape(KC, 128, H)).astype(f8).view(
            ml_dtypes.float8_e4m3fn)

    # per-j s2 scale: odd j's selu output is scaled by SW (DVE relu path)
    jscale = np.where(np.arange(JC) % 2 == 1, SW, 1.0).astype(np.float32)
    w2l = (W2[:, 0] * SELU_LAMBDA).reshape(JC, 128) / jscale[:, None]
    w2lp = np.zeros((128, JC, 32), bf16)
    w2lp[:, :, 0] = w2l.T.astype(bf16)

    # reduction masks: tg0/half0 partials live at partitions [0,32) and
    # [64,96) (real rows 0 and 64, zeros elsewhere), tg1/half1 at the
    # complement.
    m = np.zeros((128, 2), np.float32)
    m[0:32, 0] = 1.0
    m[64:96, 0] = 1.0
    m[32:64, 1] = 1.0
    m[96:128, 1] = 1.0

    # host-side hidden-state contribution: hb[b, :] = hid[b] @ W1[H:] + b1
    hb_all = hid[0] @ W1[H:] + b1                       # (B, H) f32
    ln_alpha = math.log(SELU_ALPHA)
    ln_sw = math.log(SW)

    in_maps = []
    for c in range(N_CORES):
        sl = slice(BL * c, BL * (c + 1))
        hb = hb_all[sl].reshape(BL, JC, 128).transpose(2, 1, 0)  # (128,JC,BL)
        # exp bias: hb + ln(alpha) (+ ln(SW) for odd j so e2 = SW*alpha*e^x)
        hbe = hb + ln_alpha + ln_sw * (np.arange(JC) % 2)[None, :, None]
        # relu bias: hb (ACT, even j) or SW*hb (DVE, odd j)
        hbr = hb * np.where(np.arange(JC) % 2 == 1, SW, 1.0)[None, :, None]
        in_maps.append({
            "enc": np.ascontiguousarray(enc[sl]).reshape(BL, TT, 128, H),
            "w1a8": w1a8,
            "w2lp": w2lp,
            "hbe": np.ascontiguousarray(hbe.astype(np.float32)),
            "hbr": np.ascontiguousarray(hbr.astype(np.float32)),
            "maskb": m.astype(bf16),
            "wcz": np.zeros((128, KC, 32), bf16),
        })
    return in_maps


def kernel(encoder_outputs, hidden_state, W1, b1, W2, b2):
    # b2 shifts every score equally; softmax is shift-invariant, so it is
    # deliberately unused.
    in_maps = make_in_maps(encoder_outputs, hidden_state, W1, b1, W2)
    nc = _get_nc()
    res = run_bass_kernel_spmd(nc, in_maps, core_ids=list(range(N_CORES)))
    out = np.empty((1, B, H), np.float32)
    for c in range(N_CORES):
        z = res.results[c]["zs"].sum(axis=1, keepdims=True)   # (BL, 1)
        out[0, BL * c:BL * (c + 1)] = res.results[c]["out"] / z
    return out
